# revision 7
# baseline (speedup 1.0000x reference)
"""Trainium2 Bass kernel for the NMS-detection head (nn_Baseline_16595753632199).

Reference semantics (only batch 0 reaches the output):
  heat  = sigmoid(cls_pred);  peaks = (heat == maxpool5x5(heat))
  per-class top-100 over HW, then global top-100 over [C*100]
  -> (topk_bbox [100,4], topk_score [100], topk_clses [100])

The dual top-k is exactly a global top-100 over all C*HW masked scores
(each class contributes at most 100 winners), and sigmoid is monotone, so all
ranking happens on raw logits on-device.

Device strategy (8 NeuronCores, channel-parallel over batch 0):
  * core i gets channels [10i, 10i+10) of cls_pred[0] -> SBUF [128, 5120]
  * two gpsimd.topk calls (8 tokens x 40960 vocab each) extract the top-256
    (value, index) of each 40960-element region; the <=100 global winners
    provably live inside these per-region top-256 sets.
  * each candidate's 5x5 neighbourhood is gathered from a host-padded DRAM
    copy via one indirect DMA; candidate is a peak iff value == nbhd max
    (exact f32 compare — identical to the reference's hmax==heat mask).
  * every core also decodes 1/8 of the 65536 boxes (Sigmoid/Exp on ScalarE,
    exact power-of-two affine ops on VectorE).
  * host: merges candidates, computes the reference sigmoid (jax on CPU) for
    surviving peaks, sorts by (-score, class, hw) — the reference's tie
    order — and gathers the top-100 boxes.
"""

import numpy as np

# problem geometry
B, C, H, W = 8, 80, 256, 256
NCORES = 8
CPC = C // NCORES          # channels per core
HW = H * W
SHARD = CPC * HW           # 655360 elements per core
FREE = SHARD // 128        # 5120
NTOK = 8                   # topk tokens per instruction
VOCAB = 40960              # topk vocab per token (u16 ISA field limit)
NHALF = VOCAB // 16        # 2560 free elements per partition per instruction
K = 256
PADH, PADW = H + 4, W + 4  # 260 x 260 padded channel planes
PAD_CH = PADH * PADW       # 67600
PAD_TOT = CPC * PAD_CH     # 676000
MAX_PATCH_START = (CPC - 1) * PAD_CH + (H - 1) * PADW + (W - 1)  # 674955
ROWS_PER_CORE = H // NCORES  # 32 rows of bbox decode per core
BBN = ROWS_PER_CORE * W      # 8192 boxes per core
PAD_VAL = -1.0e30
NCAND = 32                 # candidates per partition (16 per topk half)

_RUNNER = None


def _emit_topk(nc, out_ap, in_ap, tokens, vocab, k):
    """nc.gpsimd.topk without the perf-motivated vocab>50000 assert."""
    import concourse.bass_isa as bass_isa

    g = nc.gpsimd
    _in = g.lower_ap(in_ap, for_isa=True)
    _out = g.lower_ap(out_ap, for_isa=True)
    return g.add_instruction(
        bass_isa.InstTopk(
            name=f"I-{nc.next_id()}",
            ins=[_in],
            outs=[_out],
            _tokens=tokens,
            _n=vocab,
            _k=k,
        )
    )


def _build_bass():
    import concourse.bacc as bacc
    import concourse.tile as tile
    import concourse.mybir as mybir
    import concourse.bass as bass
    from concourse.bass_types import AP

    f32, u32 = mybir.dt.float32, mybir.dt.uint32
    Alu = mybir.AluOpType
    Act = mybir.ActivationFunctionType

    nc = bacc.Bacc("TRN2", target_bir_lowering=False, debug=False)

    cls_t = nc.dram_tensor("cls", [CPC, H, W], f32, kind="ExternalInput")
    pad_t = nc.dram_tensor("cls_pad", [PAD_TOT], f32, kind="ExternalInput")
    txty_t = nc.dram_tensor("txty", [2, ROWS_PER_CORE, W], f32, kind="ExternalInput")
    twth_t = nc.dram_tensor("twth", [2, ROWS_PER_CORE, W], f32, kind="ExternalInput")
    grid_t = nc.dram_tensor("grid", [2, BBN], f32, kind="ExternalInput")
    pbase_t = nc.dram_tensor("pbase", [128, 1], f32, kind="ExternalInput")
    otk_t = nc.dram_tensor("out_tk", [128, 64], u32, kind="ExternalOutput")
    opk_t = nc.dram_tensor("out_peak", [128, NCAND], f32, kind="ExternalOutput")
    obb_t = nc.dram_tensor("out_bbox", [4, BBN], f32, kind="ExternalOutput")

    def part128(ap):
        # flatten an arbitrary dram AP and view it as [128, size/128]
        src = " ".join(chr(ord("a") + i) for i in range(len(ap.shape)))
        return ap.rearrange(f"{src} -> ({src})").rearrange("(p f) -> p f", p=128)

    # raw SBUF tensors: gpsimd.topk is an ISA op needing physical addresses
    x_sb = nc.alloc_sbuf_tensor("x_sb", [128, FREE], f32)
    tk0_sb = nc.alloc_sbuf_tensor("tk0_sb", [128, 32], u32)
    tk1_sb = nc.alloc_sbuf_tensor("tk1_sb", [128, 32], u32)

    with tile.TileContext(nc) as tc:
        with tc.tile_pool(name="main", bufs=1) as pool:
            # ---- candidate extraction ----
            x = x_sb.ap()
            nc.sync.dma_start(x[:], part128(cls_t.ap()))
            tk0 = tk0_sb.ap()
            tk1 = tk1_sb.ap()
            _emit_topk(nc, tk0[:], x[:, 0:NHALF], NTOK, VOCAB, K)
            _emit_topk(nc, tk1[:], x[:, NHALF:FREE], NTOK, VOCAB, K)

            pb = pool.tile([128, 1], f32, tag="pb")
            nc.sync.dma_start(pb[:], pbase_t.ap())

            # combined per-token indices v (u32) and values (f32)
            vu = pool.tile([128, NCAND], u32, tag="vu")
            nc.vector.tensor_copy(out=vu[:, 0:16], in_=tk0[:, 16:32])
            nc.vector.tensor_copy(out=vu[:, 16:32], in_=tk1[:, 16:32])
            vals = pool.tile([128, NCAND], f32, tag="vals")
            nc.vector.tensor_copy(out=vals[:, 0:16], in_=tk0[:, 0:16].bitcast(f32))
            nc.vector.tensor_copy(out=vals[:, 16:32], in_=tk1[:, 0:16].bitcast(f32))

            # flat index within the core shard:
            #   F = v + 2560*(v//2560) + 81920*(p//16) + 2560*h
            au = pool.tile([128, NCAND], u32, tag="au")
            bu = pool.tile([128, NCAND], u32, tag="bu")
            ju = pool.tile([128, NCAND], u32, tag="ju")
            nc.vector.tensor_scalar(au[:], vu[:], 9, None, op0=Alu.logical_shift_right)
            nc.vector.tensor_scalar(bu[:], au[:], 205.0, None, op0=Alu.mult)
            nc.vector.tensor_scalar(ju[:], bu[:], 10, None, op0=Alu.logical_shift_right)
            vf = pool.tile([128, NCAND], f32, tag="vf")
            jf = pool.tile([128, NCAND], f32, tag="jf")
            nc.vector.tensor_copy(out=vf[:], in_=vu[:])
            nc.vector.tensor_copy(out=jf[:], in_=ju[:])
            Ff = pool.tile([128, NCAND], f32, tag="Ff")
            nc.vector.scalar_tensor_tensor(
                out=Ff[:], in0=jf[:], scalar=float(NHALF), in1=vf[:],
                op0=Alu.mult, op1=Alu.add,
            )
            nc.vector.tensor_scalar(Ff[:], Ff[:], pb[:], None, op0=Alu.add)
            nc.vector.tensor_scalar(
                Ff[:, 16:32], Ff[:, 16:32], float(NHALF), None, op0=Alu.add
            )

            # (c, r, w) -> offset into the padded DRAM copy
            Fu = pool.tile([128, NCAND], u32, tag="Fu")
            nc.vector.tensor_copy(out=Fu[:], in_=Ff[:])
            cch = pool.tile([128, NCAND], u32, tag="cch")
            rem = pool.tile([128, NCAND], u32, tag="rem")
            rr = pool.tile([128, NCAND], u32, tag="rr")
            ww = pool.tile([128, NCAND], u32, tag="ww")
            nc.vector.tensor_scalar(cch[:], Fu[:], 16, None, op0=Alu.logical_shift_right)
            nc.vector.tensor_scalar(rem[:], Fu[:], 65535, None, op0=Alu.bitwise_and)
            nc.vector.tensor_scalar(rr[:], rem[:], 8, None, op0=Alu.logical_shift_right)
            nc.vector.tensor_scalar(ww[:], rem[:], 255, None, op0=Alu.bitwise_and)
            cf = pool.tile([128, NCAND], f32, tag="cf")
            rf = pool.tile([128, NCAND], f32, tag="rf")
            wf = pool.tile([128, NCAND], f32, tag="wf")
            nc.vector.tensor_copy(out=cf[:], in_=cch[:])
            nc.vector.tensor_copy(out=rf[:], in_=rr[:])
            nc.vector.tensor_copy(out=wf[:], in_=ww[:])
            offf = pool.tile([128, NCAND], f32, tag="offf")
            nc.vector.tensor_scalar(offf[:], rf[:], float(PADW), None, op0=Alu.mult)
            nc.vector.scalar_tensor_tensor(
                out=offf[:], in0=cf[:], scalar=float(PAD_CH), in1=offf[:],
                op0=Alu.mult, op1=Alu.add,
            )
            nc.vector.tensor_tensor(out=offf[:], in0=offf[:], in1=wf[:], op=Alu.add)
            off = pool.tile([128, NCAND], u32, tag="off")
            nc.vector.tensor_copy(out=off[:], in_=offf[:])

            # gather the 5x5 patch around every candidate from the padded copy
            patches = pool.tile([128, NCAND * 25], f32, tag="patches")
            patch_view = AP(
                tensor=pad_t, offset=0,
                ap=[[1, MAX_PATCH_START + 1], [PADW, 5], [1, 5]],
            )
            nc.gpsimd.indirect_dma_start(
                out=patches[:],
                out_offset=None,
                in_=patch_view,
                in_offset=bass.IndirectOffsetOnAxis(ap=off[:], axis=0),
            )

            nbhd = pool.tile([128, NCAND], f32, tag="nbhd")
            nc.vector.tensor_reduce(
                nbhd[:],
                patches[:].rearrange("p (a b) -> p a b", b=25),
                axis=mybir.AxisListType.X,
                op=Alu.max,
            )
            peak = pool.tile([128, NCAND], f32, tag="peak")
            nc.vector.tensor_tensor(
                out=peak[:], in0=vals[:], in1=nbhd[:], op=Alu.is_ge
            )
            nc.sync.dma_start(otk_t.ap()[:, 0:32], tk0[:])
            nc.sync.dma_start(otk_t.ap()[:, 32:64], tk1[:])
            nc.sync.dma_start(opk_t.ap(), peak[:])

            # ---- bbox decode for this core's 8192 grid cells ----
            FB = BBN // 128  # 64
            tx = pool.tile([128, FB], f32, tag="tx")
            ty = pool.tile([128, FB], f32, tag="ty")
            tw = pool.tile([128, FB], f32, tag="tw")
            th = pool.tile([128, FB], f32, tag="th")
            gx = pool.tile([128, FB], f32, tag="gx")
            gy = pool.tile([128, FB], f32, tag="gy")
            nc.sync.dma_start(tx[:], part128(txty_t.ap()[0]))
            nc.sync.dma_start(ty[:], part128(txty_t.ap()[1]))
            nc.sync.dma_start(tw[:], part128(twth_t.ap()[0]))
            nc.sync.dma_start(th[:], part128(twth_t.ap()[1]))
            nc.sync.dma_start(gx[:], part128(grid_t.ap()[0]))
            nc.sync.dma_start(gy[:], part128(grid_t.ap()[1]))

            sx = pool.tile([128, FB], f32, tag="sx")
            sy = pool.tile([128, FB], f32, tag="sy")
            ex = pool.tile([128, FB], f32, tag="ex")
            ey = pool.tile([128, FB], f32, tag="ey")
            nc.scalar.activation(sx[:], tx[:], Act.Sigmoid)
            nc.scalar.activation(sy[:], ty[:], Act.Sigmoid)
            nc.scalar.activation(ex[:], tw[:], Act.Exp)
            nc.scalar.activation(ey[:], th[:], Act.Exp)

            ax = pool.tile([128, FB], f32, tag="ax")
            ay = pool.tile([128, FB], f32, tag="ay")
            nc.vector.tensor_tensor(out=ax[:], in0=sx[:], in1=gx[:], op=Alu.add)
            nc.vector.tensor_tensor(out=ay[:], in0=sy[:], in1=gy[:], op=Alu.add)
            # reference: xy = (grid + sig)*4 ; wh = exp*4
            #   x1 = (xy - wh/2)/1024 = (g+s)/256 - e/512   (exact pow-2 scales)
            exh = pool.tile([128, FB], f32, tag="exh")
            eyh = pool.tile([128, FB], f32, tag="eyh")
            nc.vector.tensor_scalar(exh[:], ex[:], 1.0 / 512.0, None, op0=Alu.mult)
            nc.vector.tensor_scalar(eyh[:], ey[:], 1.0 / 512.0, None, op0=Alu.mult)

            bplanes = []
            for name, acc, ehalf, opx in (
                ("bx1", ax, exh, Alu.subtract),
                ("by1", ay, eyh, Alu.subtract),
                ("bx2", ax, exh, Alu.add),
                ("by2", ay, eyh, Alu.add),
            ):
                t = pool.tile([128, FB], f32, tag=name, name=name)
                nc.vector.scalar_tensor_tensor(
                    out=t[:], in0=acc[:], scalar=1.0 / 256.0, in1=ehalf[:],
                    op0=Alu.mult, op1=opx,
                )
                nc.vector.tensor_scalar(t[:], t[:], 0.0, 1.0, op0=Alu.max, op1=Alu.min)
                bplanes.append(t)
            for k in range(4):
                nc.sync.dma_start(part128(obb_t.ap()[k]), bplanes[k][:])

    nc.compile()
    return nc


def _make_in_maps(cls_pred, txty_pred, twth_pred):
    cls0 = np.ascontiguousarray(cls_pred[0], dtype=np.float32)    # [80,256,256]
    txty0 = np.ascontiguousarray(txty_pred[0], dtype=np.float32)  # [2,256,256]
    twth0 = np.ascontiguousarray(twth_pred[0], dtype=np.float32)
    pbase = (81920.0 * (np.arange(128, dtype=np.float32) // 16)).reshape(128, 1)
    pbase = pbase.astype(np.float32)
    wcol = np.tile(np.arange(W, dtype=np.float32), ROWS_PER_CORE)
    in_maps = []
    for i in range(NCORES):
        ch = cls0[i * CPC:(i + 1) * CPC]
        pad = np.full((CPC, PADH, PADW), PAD_VAL, dtype=np.float32)
        pad[:, 2:2 + H, 2:2 + W] = ch
        r0 = i * ROWS_PER_CORE
        gy = (r0 + np.arange(BBN, dtype=np.float32) // W).astype(np.float32)
        in_maps.append({
            "cls": ch,
            "cls_pad": pad.reshape(-1),
            "txty": np.ascontiguousarray(txty0[:, r0:r0 + ROWS_PER_CORE, :]),
            "twth": np.ascontiguousarray(twth0[:, r0:r0 + ROWS_PER_CORE, :]),
            "grid": np.stack([wcol, gy]),
            "pbase": pbase,
        })
    return in_maps


def _decode_candidates(tk_half, prow, h):
    """tk_half: [128,32] u32 (16 values then 16 indices). Returns vals, F."""
    vals = tk_half[:, :16].copy().view(np.float32)
    v = tk_half[:, 16:32].astype(np.int64)
    F = v + NHALF * (v // NHALF) + 81920 * (prow // 16) + NHALF * h
    return vals, F


def _merge_outputs(results):
    """results: list of 8 dicts with out_tk/out_peak/out_bbox -> ref output."""
    import jax
    import jax.numpy as jnp

    logits, clses, hws = [], [], []
    prow = np.arange(128, dtype=np.int64)[:, None]
    for i, r in enumerate(results):
        tk = np.asarray(r["out_tk"])
        peak = np.asarray(r["out_peak"]) > 0.5
        for h in (0, 1):
            vals, F = _decode_candidates(tk[:, 32 * h:32 * h + 32], prow, h)
            pk = peak[:, 16 * h:16 * h + 16]
            c = i * CPC + F // HW
            hw = F % HW
            logits.append(vals[pk])
            clses.append(c[pk])
            hws.append(hw[pk])
    logits = np.concatenate(logits)
    clses = np.concatenate(clses)
    hws = np.concatenate(hws)

    cpu = jax.devices("cpu")[0]
    with jax.default_device(cpu):
        scores = np.asarray(jax.nn.sigmoid(jnp.asarray(logits, dtype=jnp.float32)))

    # reference tie order: score desc, then class asc, then hw asc
    order = np.lexsort((hws, clses, -scores))[:100]
    topk_score = scores[order].astype(np.float32)
    topk_clses = clses[order].astype(np.int32)
    topk_hw = hws[order]

    bbox = np.concatenate(
        [np.asarray(r["out_bbox"]) for r in results], axis=1
    )  # [4, 65536]
    topk_bbox = np.ascontiguousarray(bbox[:, topk_hw].T.astype(np.float32))
    return topk_bbox, topk_score, topk_clses


class _Runner:
    """Builds the Bass program once and keeps a persistent jitted PJRT callable."""

    def __init__(self):
        self.nc = _build_bass()
        self._sharded = None
        self._names = None

    def _setup_pjrt(self):
        import jax
        import concourse.mybir as mybir
        from jax.sharding import Mesh, PartitionSpec
        from jax.experimental.shard_map import shard_map
        from concourse import bass2jax

        nc = self.nc
        bass2jax.install_neuronx_cc_hook()
        partition_name = (
            nc.partition_id_tensor.name if nc.partition_id_tensor else None
        )
        in_names, out_names, out_avals, zero_outs = [], [], [], []
        for alloc in nc.m.functions[0].allocations:
            if not isinstance(alloc, mybir.MemoryLocationSet):
                continue
            name = alloc.memorylocations[0].name
            if alloc.kind == "ExternalInput":
                if name != partition_name:
                    in_names.append(name)
            elif alloc.kind == "ExternalOutput":
                out_names.append(name)
                shape = tuple(alloc.tensor_shape)
                dtype = mybir.dt.np(alloc.dtype)
                out_avals.append(jax.core.ShapedArray(shape, dtype))
                zero_outs.append(np.zeros(shape, dtype))
        n_params = len(in_names)
        donate = tuple(range(n_params, n_params + len(out_names)))

        bind_names = list(in_names) + list(out_names)
        if partition_name is not None:
            bind_names.append(partition_name)

        def _body(*args):
            operands = list(args)
            if partition_name is not None:
                operands.append(bass2jax.partition_id_tensor())
            outs = bass2jax._bass_exec_p.bind(
                *operands,
                out_avals=tuple(out_avals),
                in_names=tuple(bind_names),
                out_names=tuple(out_names),
                lowering_input_output_aliases=(),
                sim_require_finite=True,
                sim_require_nnan=True,
                nc=nc,
            )
            return tuple(outs)

        devices = jax.devices()[:NCORES]
        mesh = Mesh(np.asarray(devices), ("core",))
        specs = (PartitionSpec("core"),) * (n_params + len(out_names))
        self._sharded = jax.jit(
            shard_map(
                _body, mesh=mesh, in_specs=specs,
                out_specs=(PartitionSpec("core"),) * len(out_names),
                check_rep=False,
            ),
            donate_argnums=donate,
            keep_unused=True,
        )
        self._names = (in_names, out_names, out_avals, zero_outs)

    def run(self, in_maps):
        if self._sharded is None:
            self._setup_pjrt()
        in_names, out_names, out_avals, zero_outs = self._names
        concat_in = [
            np.concatenate([np.asarray(m[name]) for m in in_maps], axis=0)
            for name in in_names
        ]
        concat_zeros = [
            np.zeros((NCORES * z.shape[0], *z.shape[1:]), z.dtype) for z in zero_outs
        ]
        out_arrs = self._sharded(*concat_in, *concat_zeros)
        return [
            {
                name: np.asarray(out_arrs[j]).reshape(NCORES, *out_avals[j].shape)[c]
                for j, name in enumerate(out_names)
            }
            for c in range(NCORES)
        ]


def _get_runner():
    global _RUNNER
    if _RUNNER is None:
        _RUNNER = _Runner()
    return _RUNNER


def kernel(cls_pred, txty_pred, twth_pred):
    runner = _get_runner()
    in_maps = _make_in_maps(cls_pred, txty_pred, twth_pred)
    results = runner.run(in_maps)
    return _merge_outputs(results)


# revision 11
# speedup vs baseline: 2.2341x; 2.2341x over previous
"""Trainium2 Bass kernel for the NMS-detection head (nn_Baseline_16595753632199).

Reference semantics (only batch 0 reaches the output):
  heat  = sigmoid(cls_pred);  peaks = (heat == maxpool5x5(heat))
  per-class top-100 over HW, then global top-100 over [C*100]
  -> (topk_bbox [100,4], topk_score [100], topk_clses [100])

The dual top-k is exactly a global top-100 over all C*HW masked scores
(each class contributes at most 100 winners), and sigmoid is monotone, so all
ranking happens on raw logits on-device.

Device strategy (8 NeuronCores, channel-parallel over batch 0):
  * core i gets channels [10i, 10i+10) of cls_pred[0] -> SBUF [128, 5120]
  * two gpsimd.topk calls (8 tokens x 40960 vocab each) extract the top-256
    (value, index) of each 40960-element region; the <=100 global winners
    provably live inside these per-region top-256 sets.
  * each candidate's 5x5 neighbourhood is gathered from a host-padded DRAM
    copy via one indirect DMA; candidate is a peak iff value == nbhd max
    (exact f32 compare — identical to the reference's hmax==heat mask).
  * every core also decodes 1/8 of the 65536 boxes (Sigmoid/Exp on ScalarE,
    exact power-of-two affine ops on VectorE).
  * host: merges candidates, computes the reference sigmoid (jax on CPU) for
    surviving peaks, sorts by (-score, class, hw) — the reference's tie
    order — and gathers the top-100 boxes.
"""

import numpy as np

# problem geometry
B, C, H, W = 8, 80, 256, 256
NCORES = 8
CPC = C // NCORES          # channels per core
HW = H * W
SHARD = CPC * HW           # 655360 elements per core
FREE = SHARD // 128        # 5120
NTOK = 8                   # topk tokens per instruction
VOCAB = 40960              # topk vocab per token (u16 ISA field limit)
NHALF = VOCAB // 16        # 2560 free elements per partition per instruction
K = 256
PADH, PADW = H + 4, W + 4  # 260 x 260 padded channel planes
PAD_CH = PADH * PADW       # 67600
PAD_TOT = CPC * PAD_CH     # 676000
MAX_PATCH_START = (CPC - 1) * PAD_CH + (H - 1) * PADW + (W - 1)  # 674955
ROWS_PER_CORE = H // NCORES  # 32 rows of bbox decode per core
BBN = ROWS_PER_CORE * W      # 8192 boxes per core
PAD_VAL = -1.0e30
NCAND = 32                 # candidates per partition (16 per topk half)
OUT_ELEMS = 4096 * 3 + 4 * 8192  # packed flat output (f32 elements)

_RUNNER = None


def _emit_topk(nc, out_ap, in_ap, tokens, vocab, k):
    """nc.gpsimd.topk without the perf-motivated vocab>50000 assert."""
    import concourse.bass_isa as bass_isa

    g = nc.gpsimd
    _in = g.lower_ap(in_ap, for_isa=True)
    _out = g.lower_ap(out_ap, for_isa=True)
    return g.add_instruction(
        bass_isa.InstTopk(
            name=f"I-{nc.next_id()}",
            ins=[_in],
            outs=[_out],
            _tokens=tokens,
            _n=vocab,
            _k=k,
        )
    )


def _build_bass():
    import concourse.bacc as bacc
    import concourse.tile as tile
    import concourse.mybir as mybir
    import concourse.bass as bass
    from concourse.bass_types import AP

    f32, u32 = mybir.dt.float32, mybir.dt.uint32
    Alu = mybir.AluOpType
    Act = mybir.ActivationFunctionType

    nc = bacc.Bacc("TRN2", target_bir_lowering=False, debug=False)

    pad_t = nc.dram_tensor("cls_pad", [CPC, PADH, PADW], f32, kind="ExternalInput")
    txty_t = nc.dram_tensor("txty", [2, ROWS_PER_CORE, W], f32, kind="ExternalInput")
    twth_t = nc.dram_tensor("twth", [2, ROWS_PER_CORE, W], f32, kind="ExternalInput")
    grid_t = nc.dram_tensor("grid", [2, BBN], f32, kind="ExternalInput")
    pbase_t = nc.dram_tensor("pbase", [128, 1], f32, kind="ExternalInput")
    out_t = nc.dram_tensor("out_flat", [OUT_ELEMS], f32, kind="ExternalOutput")

    def part128(ap):
        # flatten an arbitrary dram AP and view it as [128, size/128]
        src = " ".join(chr(ord("a") + i) for i in range(len(ap.shape)))
        return ap.rearrange(f"{src} -> ({src})").rearrange("(p f) -> p f", p=128)

    # raw SBUF tensors: gpsimd.topk is an ISA op needing physical addresses
    x_sb = nc.alloc_sbuf_tensor("x_sb", [128, FREE], f32)
    tk0_sb = nc.alloc_sbuf_tensor("tk0_sb", [128, 32], u32)
    tk1_sb = nc.alloc_sbuf_tensor("tk1_sb", [128, 32], u32)

    with tile.TileContext(nc) as tc:
        with tc.tile_pool(name="main", bufs=1) as pool:
            # ---- candidate extraction ----
            x = x_sb.ap()
            x_src = AP(
                tensor=pad_t, offset=2 * PADW + 2,
                ap=[[PAD_CH, CPC], [PADW, H], [1, W]],
            )
            nc.sync.dma_start(x[:], x_src)
            tk0 = tk0_sb.ap()
            tk1 = tk1_sb.ap()
            _emit_topk(nc, tk0[:], x[:, 0:NHALF], NTOK, VOCAB, K)
            _emit_topk(nc, tk1[:], x[:, NHALF:FREE], NTOK, VOCAB, K)

            pb = pool.tile([128, 1], f32, tag="pb")
            nc.sync.dma_start(pb[:], pbase_t.ap())

            # combined per-token indices v (u32) and values (f32)
            vu = pool.tile([128, NCAND], u32, tag="vu")
            nc.vector.tensor_copy(out=vu[:, 0:16], in_=tk0[:, 16:32])
            nc.vector.tensor_copy(out=vu[:, 16:32], in_=tk1[:, 16:32])
            vals = pool.tile([128, NCAND], f32, tag="vals")
            nc.vector.tensor_copy(out=vals[:, 0:16], in_=tk0[:, 0:16].bitcast(f32))
            nc.vector.tensor_copy(out=vals[:, 16:32], in_=tk1[:, 0:16].bitcast(f32))

            # flat index within the core shard:
            #   F = v + 2560*(v//2560) + 81920*(p//16) + 2560*h
            au = pool.tile([128, NCAND], u32, tag="au")
            bu = pool.tile([128, NCAND], u32, tag="bu")
            ju = pool.tile([128, NCAND], u32, tag="ju")
            nc.vector.tensor_scalar(au[:], vu[:], 9, None, op0=Alu.logical_shift_right)
            nc.vector.tensor_scalar(bu[:], au[:], 205.0, None, op0=Alu.mult)
            nc.vector.tensor_scalar(ju[:], bu[:], 10, None, op0=Alu.logical_shift_right)
            vf = pool.tile([128, NCAND], f32, tag="vf")
            jf = pool.tile([128, NCAND], f32, tag="jf")
            nc.vector.tensor_copy(out=vf[:], in_=vu[:])
            nc.vector.tensor_copy(out=jf[:], in_=ju[:])
            Ff = pool.tile([128, NCAND], f32, tag="Ff")
            nc.vector.scalar_tensor_tensor(
                out=Ff[:], in0=jf[:], scalar=float(NHALF), in1=vf[:],
                op0=Alu.mult, op1=Alu.add,
            )
            nc.vector.tensor_scalar(Ff[:], Ff[:], pb[:], None, op0=Alu.add)
            nc.vector.tensor_scalar(
                Ff[:, 16:32], Ff[:, 16:32], float(NHALF), None, op0=Alu.add
            )

            # (c, r, w) -> offset into the padded DRAM copy
            Fu = pool.tile([128, NCAND], u32, tag="Fu")
            nc.vector.tensor_copy(out=Fu[:], in_=Ff[:])
            cch = pool.tile([128, NCAND], u32, tag="cch")
            rem = pool.tile([128, NCAND], u32, tag="rem")
            rr = pool.tile([128, NCAND], u32, tag="rr")
            ww = pool.tile([128, NCAND], u32, tag="ww")
            nc.vector.tensor_scalar(cch[:], Fu[:], 16, None, op0=Alu.logical_shift_right)
            nc.vector.tensor_scalar(rem[:], Fu[:], 65535, None, op0=Alu.bitwise_and)
            nc.vector.tensor_scalar(rr[:], rem[:], 8, None, op0=Alu.logical_shift_right)
            nc.vector.tensor_scalar(ww[:], rem[:], 255, None, op0=Alu.bitwise_and)
            cf = pool.tile([128, NCAND], f32, tag="cf")
            rf = pool.tile([128, NCAND], f32, tag="rf")
            wf = pool.tile([128, NCAND], f32, tag="wf")
            nc.vector.tensor_copy(out=cf[:], in_=cch[:])
            nc.vector.tensor_copy(out=rf[:], in_=rr[:])
            nc.vector.tensor_copy(out=wf[:], in_=ww[:])
            offf = pool.tile([128, NCAND], f32, tag="offf")
            nc.vector.tensor_scalar(offf[:], rf[:], float(PADW), None, op0=Alu.mult)
            nc.vector.scalar_tensor_tensor(
                out=offf[:], in0=cf[:], scalar=float(PAD_CH), in1=offf[:],
                op0=Alu.mult, op1=Alu.add,
            )
            nc.vector.tensor_tensor(out=offf[:], in0=offf[:], in1=wf[:], op=Alu.add)
            off = pool.tile([128, NCAND], u32, tag="off")
            nc.vector.tensor_copy(out=off[:], in_=offf[:])

            # gather the 5x5 patch around every candidate from the padded copy
            patches = pool.tile([128, NCAND * 25], f32, tag="patches")
            patch_view = AP(
                tensor=pad_t, offset=0,
                ap=[[1, MAX_PATCH_START + 1], [PADW, 5], [1, 5]],
            )
            nc.gpsimd.indirect_dma_start(
                out=patches[:],
                out_offset=None,
                in_=patch_view,
                in_offset=bass.IndirectOffsetOnAxis(ap=off[:], axis=0),
            )

            nbhd = pool.tile([128, NCAND], f32, tag="nbhd")
            nc.vector.tensor_reduce(
                nbhd[:],
                patches[:].rearrange("p (a b) -> p a b", b=25),
                axis=mybir.AxisListType.X,
                op=Alu.max,
            )
            peak = pool.tile([128, NCAND], f32, tag="peak")
            nc.vector.tensor_tensor(
                out=peak[:], in0=vals[:], in1=nbhd[:], op=Alu.is_ge
            )
            of = out_t.ap()
            nc.sync.dma_start(
                of[0:4096].rearrange("(p f) -> p f", p=128), tk0[:].bitcast(f32))
            nc.sync.dma_start(
                of[4096:8192].rearrange("(p f) -> p f", p=128), tk1[:].bitcast(f32))
            nc.sync.dma_start(
                of[8192:12288].rearrange("(p f) -> p f", p=128), peak[:])

            # ---- bbox decode for this core's 8192 grid cells ----
            FB = BBN // 128  # 64
            tx = pool.tile([128, FB], f32, tag="tx")
            ty = pool.tile([128, FB], f32, tag="ty")
            tw = pool.tile([128, FB], f32, tag="tw")
            th = pool.tile([128, FB], f32, tag="th")
            gx = pool.tile([128, FB], f32, tag="gx")
            gy = pool.tile([128, FB], f32, tag="gy")
            nc.sync.dma_start(tx[:], part128(txty_t.ap()[0]))
            nc.sync.dma_start(ty[:], part128(txty_t.ap()[1]))
            nc.sync.dma_start(tw[:], part128(twth_t.ap()[0]))
            nc.sync.dma_start(th[:], part128(twth_t.ap()[1]))
            nc.sync.dma_start(gx[:], part128(grid_t.ap()[0]))
            nc.sync.dma_start(gy[:], part128(grid_t.ap()[1]))

            sx = pool.tile([128, FB], f32, tag="sx")
            sy = pool.tile([128, FB], f32, tag="sy")
            ex = pool.tile([128, FB], f32, tag="ex")
            ey = pool.tile([128, FB], f32, tag="ey")
            nc.scalar.activation(sx[:], tx[:], Act.Sigmoid)
            nc.scalar.activation(sy[:], ty[:], Act.Sigmoid)
            nc.scalar.activation(ex[:], tw[:], Act.Exp)
            nc.scalar.activation(ey[:], th[:], Act.Exp)

            ax = pool.tile([128, FB], f32, tag="ax")
            ay = pool.tile([128, FB], f32, tag="ay")
            nc.vector.tensor_tensor(out=ax[:], in0=sx[:], in1=gx[:], op=Alu.add)
            nc.vector.tensor_tensor(out=ay[:], in0=sy[:], in1=gy[:], op=Alu.add)
            # reference: xy = (grid + sig)*4 ; wh = exp*4
            #   x1 = (xy - wh/2)/1024 = (g+s)/256 - e/512   (exact pow-2 scales)
            exh = pool.tile([128, FB], f32, tag="exh")
            eyh = pool.tile([128, FB], f32, tag="eyh")
            nc.vector.tensor_scalar(exh[:], ex[:], 1.0 / 512.0, None, op0=Alu.mult)
            nc.vector.tensor_scalar(eyh[:], ey[:], 1.0 / 512.0, None, op0=Alu.mult)

            bplanes = []
            for name, acc, ehalf, opx in (
                ("bx1", ax, exh, Alu.subtract),
                ("by1", ay, eyh, Alu.subtract),
                ("bx2", ax, exh, Alu.add),
                ("by2", ay, eyh, Alu.add),
            ):
                t = pool.tile([128, FB], f32, tag=name, name=name)
                nc.vector.scalar_tensor_tensor(
                    out=t[:], in0=acc[:], scalar=1.0 / 256.0, in1=ehalf[:],
                    op0=Alu.mult, op1=opx,
                )
                nc.vector.tensor_scalar(t[:], t[:], 0.0, 1.0, op0=Alu.max, op1=Alu.min)
                bplanes.append(t)
            for k in range(4):
                nc.sync.dma_start(
                    of[12288 + BBN * k:12288 + BBN * (k + 1)].rearrange(
                        "(p f) -> p f", p=128),
                    bplanes[k][:])

    nc.compile()
    return nc


def _make_in_maps(cls_pred, txty_pred, twth_pred):
    cls0 = np.ascontiguousarray(cls_pred[0], dtype=np.float32)    # [80,256,256]
    txty0 = np.ascontiguousarray(txty_pred[0], dtype=np.float32)  # [2,256,256]
    twth0 = np.ascontiguousarray(twth_pred[0], dtype=np.float32)
    pbase = (81920.0 * (np.arange(128, dtype=np.float32) // 16)).reshape(128, 1)
    pbase = pbase.astype(np.float32)
    wcol = np.tile(np.arange(W, dtype=np.float32), ROWS_PER_CORE)
    in_maps = []
    for i in range(NCORES):
        ch = cls0[i * CPC:(i + 1) * CPC]
        pad = np.full((CPC, PADH, PADW), PAD_VAL, dtype=np.float32)
        pad[:, 2:2 + H, 2:2 + W] = ch
        r0 = i * ROWS_PER_CORE
        gy = (r0 + np.arange(BBN, dtype=np.float32) // W).astype(np.float32)
        in_maps.append({
            "cls_pad": pad,
            "txty": np.ascontiguousarray(txty0[:, r0:r0 + ROWS_PER_CORE, :]),
            "twth": np.ascontiguousarray(twth0[:, r0:r0 + ROWS_PER_CORE, :]),
            "grid": np.stack([wcol, gy]),
            "pbase": pbase,
        })
    return in_maps


def _decode_candidates(tk_half, prow, h):
    """tk_half: [128,32] u32 (16 values then 16 indices). Returns vals, F."""
    vals = tk_half[:, :16].copy().view(np.float32)
    v = tk_half[:, 16:32].astype(np.int64)
    F = v + NHALF * (v // NHALF) + 81920 * (prow // 16) + NHALF * h
    return vals, F


def _merge_outputs(results):
    """results: list of 8 dicts with out_tk/out_peak/out_bbox -> ref output."""
    import jax
    import jax.numpy as jnp

    logits, clses, hws = [], [], []
    prow = np.arange(128, dtype=np.int64)[:, None]
    for i, r in enumerate(results):
        flat = np.asarray(r["out_flat"])
        tk = np.concatenate(
            [flat[0:4096].reshape(128, 32), flat[4096:8192].reshape(128, 32)],
            axis=1).view(np.uint32)
        peak = flat[8192:12288].reshape(128, 32) > 0.5
        for h in (0, 1):
            vals, F = _decode_candidates(tk[:, 32 * h:32 * h + 32], prow, h)
            pk = peak[:, 16 * h:16 * h + 16]
            c = i * CPC + F // HW
            hw = F % HW
            logits.append(vals[pk])
            clses.append(c[pk])
            hws.append(hw[pk])
    logits = np.concatenate(logits)
    clses = np.concatenate(clses)
    hws = np.concatenate(hws)

    cpu = jax.devices("cpu")[0]
    with jax.default_device(cpu):
        scores = np.asarray(jax.nn.sigmoid(jnp.asarray(logits, dtype=jnp.float32)))

    # reference tie order: score desc, then class asc, then hw asc
    order = np.lexsort((hws, clses, -scores))[:100]
    topk_score = scores[order].astype(np.float32)
    topk_clses = clses[order].astype(np.int32)
    topk_hw = hws[order]

    bbox = np.concatenate(
        [np.asarray(r["out_flat"])[12288:].reshape(4, BBN) for r in results], axis=1
    )  # [4, 65536]
    topk_bbox = np.ascontiguousarray(bbox[:, topk_hw].T.astype(np.float32))
    return topk_bbox, topk_score, topk_clses


class _Runner:
    """Builds the Bass program once and keeps a persistent jitted PJRT callable."""

    def __init__(self):
        self.nc = _build_bass()
        self._sharded = None
        self._names = None

    def _setup_pjrt(self):
        import jax
        import concourse.mybir as mybir
        from jax.sharding import Mesh, PartitionSpec
        from jax.experimental.shard_map import shard_map
        from concourse import bass2jax

        nc = self.nc
        bass2jax.install_neuronx_cc_hook()
        partition_name = (
            nc.partition_id_tensor.name if nc.partition_id_tensor else None
        )
        in_names, out_names, out_avals, zero_outs = [], [], [], []
        for alloc in nc.m.functions[0].allocations:
            if not isinstance(alloc, mybir.MemoryLocationSet):
                continue
            name = alloc.memorylocations[0].name
            if alloc.kind == "ExternalInput":
                if name != partition_name:
                    in_names.append(name)
            elif alloc.kind == "ExternalOutput":
                out_names.append(name)
                shape = tuple(alloc.tensor_shape)
                dtype = mybir.dt.np(alloc.dtype)
                out_avals.append(jax.core.ShapedArray(shape, dtype))
                zero_outs.append(np.zeros(shape, dtype))
        n_params = len(in_names)
        donate = tuple(range(n_params, n_params + len(out_names)))

        bind_names = list(in_names) + list(out_names)
        if partition_name is not None:
            bind_names.append(partition_name)

        def _body(*args):
            operands = list(args)
            if partition_name is not None:
                operands.append(bass2jax.partition_id_tensor())
            outs = bass2jax._bass_exec_p.bind(
                *operands,
                out_avals=tuple(out_avals),
                in_names=tuple(bind_names),
                out_names=tuple(out_names),
                lowering_input_output_aliases=(),
                sim_require_finite=True,
                sim_require_nnan=True,
                nc=nc,
            )
            return tuple(outs)

        devices = jax.devices()[:NCORES]
        mesh = Mesh(np.asarray(devices), ("core",))
        specs = (PartitionSpec("core"),) * (n_params + len(out_names))
        self._sharded = jax.jit(
            shard_map(
                _body, mesh=mesh, in_specs=specs,
                out_specs=(PartitionSpec("core"),) * len(out_names),
                check_rep=False,
            ),
            keep_unused=True,
        )
        self._names = (in_names, out_names, out_avals, zero_outs)
        from jax.sharding import NamedSharding
        sh = NamedSharding(mesh, PartitionSpec("core"))
        self._dev_zeros = [
            jax.device_put(
                np.zeros((NCORES * z.shape[0], *z.shape[1:]), z.dtype), sh
            )
            for z in zero_outs
        ]

    def run(self, in_maps):
        if self._sharded is None:
            self._setup_pjrt()
        in_names, out_names, out_avals, zero_outs = self._names
        concat_in = [
            np.concatenate([np.asarray(m[name]) for m in in_maps], axis=0)
            for name in in_names
        ]
        out_arrs = self._sharded(*concat_in, *self._dev_zeros)
        return [
            {
                name: np.asarray(out_arrs[j]).reshape(NCORES, *out_avals[j].shape)[c]
                for j, name in enumerate(out_names)
            }
            for c in range(NCORES)
        ]


def _get_runner():
    global _RUNNER
    if _RUNNER is None:
        _RUNNER = _Runner()
    return _RUNNER


def kernel(cls_pred, txty_pred, twth_pred):
    from concourse._compat import axon_active

    runner = _get_runner()
    in_maps = _make_in_maps(cls_pred, txty_pred, twth_pred)
    if axon_active():
        results = runner.run(in_maps)
    else:
        from concourse import bass_utils

        results = bass_utils.run_bass_kernel_spmd(
            runner.nc, in_maps, core_ids=list(range(NCORES))
        ).results
    return _merge_outputs(results)
